# revision 1
# baseline (speedup 1.0000x reference)
"""PALU low-rank Llama attention on 8 Trainium2 NeuronCores.

Tensor-parallel over the 8 PALU groups (1 group = 4 heads per core), with each
core computing a partial contribution to the full output (its group's slice of
the fused Wo contraction); the host sums the 8 partials.

Per-core pipeline (one Bass/Tile kernel, SPMD over 8 cores with per-core
weight shards as inputs):
  A) projections: Q (RoPE'd, pre-scaled 1/sqrt(d)), K (low-rank reconstructed
     + RoPE), V latent.  Q/K transposed to (d, s) layout via TensorE and
     spilled to DRAM scratch; V spilled natural (s, f').
  B) attention per (batch, head): scores computed transposed S.T = K.T' Q
     (k on partitions), causal tiles only, mask added on diagonal 128x128
     subtiles, exp on ScalarE straight out of PSUM, then O = P.T' V with an
     extra all-ones V column producing the softmax denominators for free.
     O is normalized with per-partition reciprocal scalars and transposed
     into O.T buffers for stage C.
  C) fused output projection with the exact torch reshape/transpose
     semantics folded into strided access patterns on O.T.

All matmuls run in float32r (TensorE full-rate fp32, ~11 mantissa bits of
operand precision); measured end-to-end error vs the fp32 reference is
~2.6e-4 relative to the output absmax.
"""

import numpy as np

import concourse.bass as bass
import concourse.tile as tile
from concourse import bacc, mybir
from concourse.bass_utils import run_bass_kernel_spmd
from concourse.masks import make_identity

F32 = mybir.dt.float32
F32R = mybir.dt.float32r
EXP = mybir.ActivationFunctionType.Exp

B, S, HID = 2, 2048, 4096
NH, D = 32, 128
G, GS = 8, 4
RK, RV, FGD, GD = 256, 2048, 256, 512
NCORES = 8
CT = HID // 128        # 32 contraction tiles over hidden dim
NQC = S // 512         # 4 q-chunks of 512
THETA = 10000.0

_NC_CACHE = {}


def _install_loud_cc_hook():
    """Surface exceptions thrown inside the neuronx_cc compile hook (the C++
    callback boundary otherwise swallows them into an opaque INTERNAL error)."""
    if _NC_CACHE.get("loud_hook"):
        return
    import traceback
    from concourse import bass2jax
    orig = bass2jax.neuronx_cc_hook

    def loud_hook(*a, **kw):
        try:
            return orig(*a, **kw)
        except BaseException:
            traceback.print_exc()
            raise

    bass2jax.neuronx_cc_hook = loud_hook
    _NC_CACHE["loud_hook"] = True


def _copy(eng_nc, dst, src):
    """Engine-agnostic copy: ScalarE uses activation-Copy, VectorE tensor_copy."""
    if hasattr(eng_nc, "tensor_copy"):
        eng_nc.tensor_copy(dst, src)
    else:
        eng_nc.copy(dst, src)



def _build_nc():
    """Build + compile the per-core Bass kernel (same NEFF for all cores)."""
    nc = bacc.Bacc(trn_type="TRN2", target_bir_lowering=False, debug=False)

    hid_d = nc.dram_tensor("hidT", [B, HID, S], F32R, kind="ExternalInput").ap()
    wq_d = nc.dram_tensor("wqT", [HID, GD], F32R, kind="ExternalInput").ap()
    wk_d = nc.dram_tensor("wkT", [HID, RK], F32R, kind="ExternalInput").ap()
    wv_d = nc.dram_tensor("wvT", [HID, FGD], F32R, kind="ExternalInput").ap()
    ug_d = nc.dram_tensor("ugT", [RK, GD], F32R, kind="ExternalInput").ap()
    cos_d = nc.dram_tensor("cos4", [B, S, GD], F32, kind="ExternalInput").ap()
    sin_d = nc.dram_tensor("sin4", [B, S, GD], F32, kind="ExternalInput").ap()
    msk_d = nc.dram_tensor("maskT", [B, S // 128, 128, 128], F32,
                           kind="ExternalInput").ap()
    wo_d = nc.dram_tensor("woT", [4, 2, 128, HID], F32R, kind="ExternalInput").ap()
    out_d = nc.dram_tensor("out", [B, S, HID], F32, kind="ExternalOutput").ap()

    # DRAM scratch (per-core internal)
    qT_s = nc.dram_tensor("qT_s", [B, GS, 128, S], F32R).ap()
    kT_s = nc.dram_tensor("kT_s", [B, GS, 128, S], F32R).ap()
    v_s = nc.dram_tensor("v_s", [B, S, FGD], F32R).ap()

    with tile.TileContext(nc) as tc:
        # ---------- constants ----------
        with tc.tile_pool(name="const", bufs=1) as pc:
            ident = pc.tile([128, 128], F32)
            make_identity(nc, ident)

            # ================= Phase A: projections =================
            with tc.tile_pool(name="wts", bufs=1) as pw, \
                 tc.tile_pool(name="ht", bufs=6) as pht, \
                 tc.tile_pool(name="tab", bufs=2) as ptab, \
                 tc.tile_pool(name="rope", bufs=3) as prope, \
                 tc.tile_pool(name="aevac", bufs=3) as paev, \
                 tc.tile_pool(name="apsum", bufs=1, space="PSUM") as pps:

                wq_sb = pw.tile([128, CT * GD], F32R, tag="wq")
                wk_sb = pw.tile([128, CT * RK], F32R, tag="wk")
                wv_sb = pw.tile([128, CT * FGD], F32R, tag="wv")
                ug_sb = pw.tile([128, 2 * GD], F32R, tag="ug")
                for ct in range(CT):
                    nc.sync.dma_start(wq_sb[:, ct * GD:(ct + 1) * GD],
                                        wq_d[ct * 128:(ct + 1) * 128, :])
                    nc.sync.dma_start(wk_sb[:, ct * RK:(ct + 1) * RK],
                                        wk_d[ct * 128:(ct + 1) * 128, :])
                    nc.sync.dma_start(wv_sb[:, ct * FGD:(ct + 1) * FGD],
                                        wv_d[ct * 128:(ct + 1) * 128, :])
                for rt in range(2):
                    nc.sync.dma_start(ug_sb[:, rt * GD:(rt + 1) * GD],
                                        ug_d[rt * 128:(rt + 1) * 128, :])

                def rope_and_spill(xp, cos_t, sin_t, dst, b, qc, mt, eng):
                    """xp: PSUM (128,512) natural (q, 4*d). RoPE -> transpose
                    per head -> evacuate -> DMA to dst[b, h][:, qc*512+mt*128]."""
                    tmp = prope.tile([128, GD], F32, tag="tmp")
                    rot = prope.tile([128, GD], F32, tag="rot")
                    xr = xp[:].rearrange("p (h t d) -> p h t d", t=2, d=64)
                    rr = rot[:].rearrange("p (h t d) -> p h t d", t=2, d=64)
                    sr = sin_t[:].rearrange("p (h t d) -> p h t d", t=2, d=64)
                    nc.vector.tensor_mul(tmp[:], xp[:], cos_t[:])
                    nc.vector.tensor_mul(rr[:, :, 0], xr[:, :, 1], sr[:, :, 0])
                    nc.vector.tensor_mul(rr[:, :, 1], xr[:, :, 0], sr[:, :, 1])
                    xn = prope.tile([128, GD], F32, tag="xn")
                    nc.vector.tensor_add(xn[:], tmp[:], rot[:])
                    tp = pps.tile([128, GD], F32, tag="ps", bufs=8)
                    for h in range(GS):
                        nc.tensor.transpose(tp[:, h * 128:(h + 1) * 128],
                                            xn[:, h * 128:(h + 1) * 128],
                                            ident[:])
                    ev = paev.tile([128, GD], F32R, tag="ev")
                    _copy(eng, ev[:], tp[:])
                    for h in range(GS):
                        nc.sync.dma_start(
                            dst[b, h, :, qc * 512 + mt * 128:
                                qc * 512 + (mt + 1) * 128],
                            ev[:, h * 128:(h + 1) * 128])

                for b in range(B):
                    for qc in range(NQC):
                        # All 8 PSUM banks accumulate through the ct loop:
                        # klat.T (2) + V-packed (2) + Q natural (4).
                        klp = [pps.tile([128, 512], F32, tag="ps", bufs=8,
                                        name=f"klp{rt}") for rt in range(2)]
                        vp = [pps.tile([128, 512], F32, tag="ps", bufs=8,
                                       name=f"vp{i}") for i in range(2)]
                        qp = [pps.tile([128, GD], F32, tag="ps", bufs=8,
                                       name=f"qp{mt}") for mt in range(4)]
                        for ct in range(CT):
                            t = pht.tile([128, 512], F32R, tag="ht")
                            nc.sync.dma_start(
                                t[:], hid_d[b, ct * 128:(ct + 1) * 128,
                                            qc * 512:(qc + 1) * 512])
                            st, sp = (ct == 0), (ct == CT - 1)
                            for rt in range(2):
                                nc.tensor.matmul(
                                    klp[rt][:],
                                    wk_sb[:, ct * RK + rt * 128:
                                          ct * RK + (rt + 1) * 128],
                                    t[:], start=st, stop=sp)
                            # V transposed: (f'-tile, 512 q), one group per bank
                            for fp in range(2):
                                nc.tensor.matmul(
                                    vp[fp][:],
                                    wv_sb[:, ct * FGD + fp * 128:
                                          ct * FGD + (fp + 1) * 128],
                                    t[:], start=st, stop=sp)
                            for mt in range(4):
                                nc.tensor.matmul(
                                    qp[mt][:],
                                    t[:, mt * 128:(mt + 1) * 128],
                                    wq_sb[:, ct * GD:(ct + 1) * GD],
                                    start=st, stop=sp)

                        # evacuate K-latent + V.T (frees 4 banks)
                        klat_sb = []
                        for rt in range(2):
                            ksb = paev.tile([128, 512], F32R, tag="klsb",
                                            name=f"ksb{rt}")
                            nc.scalar.copy(ksb[:], klp[rt][:])
                            klat_sb.append(ksb)
                        vt_sb = []
                        for fp in range(2):
                            vtsb = paev.tile([128, 512], F32, tag="vtsb",
                                             name=f"vtsb{fp}")
                            nc.vector.tensor_copy(vtsb[:], vp[fp][:])
                            vt_sb.append(vtsb)
                        # transpose V.T -> V natural (q, 256) and spill
                        for i in range(2):
                            vn = pps.tile([128, 512], F32, tag="ps", bufs=8,
                                          name=f"vn{i}")
                            for half in range(2):
                                sub = 2 * i + half
                                for fp in range(2):
                                    nc.tensor.transpose(
                                        vn[:, half * FGD + fp * 128:
                                           half * FGD + (fp + 1) * 128],
                                        vt_sb[fp][:, sub * 128:(sub + 1) * 128],
                                        ident[:])
                            vev = paev.tile([128, 512], F32R, tag="vev")
                            nc.vector.tensor_copy(vev[:], vn[:])
                            for half in range(2):
                                sub = 2 * i + half
                                nc.sync.dma_start(
                                    v_s[b, (qc * 4 + sub) * 128:
                                        (qc * 4 + sub + 1) * 128, :],
                                    vev[:, half * FGD:(half + 1) * FGD])

                        for mt in range(4):
                            qtile = qc * 4 + mt
                            cos_t = ptab.tile([128, GD], F32, tag="cos")
                            sin_t = ptab.tile([128, GD], F32, tag="sin")
                            nc.sync.dma_start(
                                cos_t[:], cos_d[b, qtile * 128:(qtile + 1) * 128, :])
                            nc.sync.dma_start(
                                sin_t[:], sin_d[b, qtile * 128:(qtile + 1) * 128, :])

                            rope_and_spill(qp[mt], cos_t, sin_t, qT_s, b, qc,
                                           mt, nc.vector)

                            # K natural from latent: (q, 4*d)
                            kp = pps.tile([128, GD], F32, tag="ps", bufs=8)
                            for rt in range(2):
                                nc.tensor.matmul(
                                    kp[:],
                                    klat_sb[rt][:, mt * 128:(mt + 1) * 128],
                                    ug_sb[:, rt * GD:(rt + 1) * GD],
                                    start=(rt == 0), stop=(rt == 1))
                            rope_and_spill(kp, cos_t, sin_t, kT_s, b, qc, mt,
                                           nc.scalar)

            # ================= Phase B: attention =================
            with tc.tile_pool(name="otb", bufs=1) as potb:
                ot = {}
                for b in range(B):
                    for hl in range(GS):
                        for fp in range(2):
                            ot[(b, hl, fp)] = potb.tile(
                                [128, S], F32R, name=f"ot{b}{hl}{fp}",
                                tag=f"ot{b}{hl}{fp}")

                with tc.tile_pool(name="vtb", bufs=1) as pvt, \
                     tc.tile_pool(name="mkb", bufs=16) as pmk, \
                     tc.tile_pool(name="ktb", bufs=2) as pkt, \
                     tc.tile_pool(name="qtb", bufs=2) as pqt, \
                     tc.tile_pool(name="ptb", bufs=2) as ppt, \
                     tc.tile_pool(name="bsm", bufs=4) as pbs, \
                     tc.tile_pool(name="bpsum", bufs=1, space="PSUM") as bps:

                    for b in range(B):
                        vt = []
                        for kt in range(S // 128):
                            v_t = pvt.tile([128, FGD + 2], F32R,
                                           name=f"v{b}_{kt}", tag=f"v{b}_{kt}")
                            nc.sync.dma_start(
                                v_t[:, :FGD],
                                v_s[b, kt * 128:(kt + 1) * 128, :])
                            nc.vector.memset(
                                v_t[:, FGD:FGD + 2].bitcast(F32), 1.0)
                            vt.append(v_t)
                        mk = []
                        for qt in range(S // 128):
                            m_t = pmk.tile([128, 128], F32,
                                           name=f"m{b}_{qt}", tag="mk")
                            nc.sync.dma_start(m_t[:], msk_d[b, qt])
                            mk.append(m_t)

                        for hl in range(GS):
                            kt_sb = pkt.tile([128, S], F32R, tag="kt")
                            nc.sync.dma_start(kt_sb[:], kT_s[b, hl])
                            for qc in range(NQC):
                                qt_sb = pqt.tile([128, 512], F32R, tag="qt")
                                nc.sync.dma_start(
                                    qt_sb[:],
                                    qT_s[b, hl, :, qc * 512:(qc + 1) * 512])
                                o_ps = [bps.tile([128, FGD + 2], F32, tag="ob",
                                                 bufs=4, name=f"ob{i}")
                                        for i in range(4)]
                                nkt = 4 * qc + 4
                                for g2 in range(0, nkt, 2):
                                    kts = [g2, g2 + 1]
                                    sc = bps.tile([128, 1024], F32, tag="sc", bufs=2)
                                    for i, kt in enumerate(kts):
                                        nc.tensor.matmul(
                                            sc[:, i * 512:(i + 1) * 512],
                                            kt_sb[:, kt * 128:(kt + 1) * 128],
                                            qt_sb[:],
                                            start=True, stop=True)
                                        if kt >= 4 * qc:
                                            sub = kt - 4 * qc
                                            sl = sc[:, i * 512 + sub * 128:
                                                    i * 512 + (sub + 1) * 128]
                                            nc.vector.tensor_add(
                                                sl, sl, mk[kt][:])
                                    pt = ppt.tile([128, 1024], F32R, tag="pt")
                                    nc.scalar.activation(pt[:], sc[:], EXP)
                                    for i, kt in enumerate(kts):
                                        for sub in range(4):
                                            if kt <= 4 * qc + sub:
                                                nc.tensor.matmul(
                                                    o_ps[sub][:],
                                                    pt[:, i * 512 + sub * 128:
                                                       i * 512 + (sub + 1) * 128],
                                                    vt[kt][:],
                                                    start=(kt == 0),
                                                    stop=(kt == 4 * qc + sub))
                                recip = pbs.tile([128, 4], F32, tag="rc")
                                for sub in range(4):
                                    nc.vector.reciprocal(
                                        recip[:, sub:sub + 1],
                                        o_ps[sub][:, FGD:FGD + 1])
                                for sub in range(4):
                                    o_sb = pbs.tile([128, FGD], F32, tag="osb")
                                    nc.vector.tensor_scalar_mul(
                                        o_sb[:], o_ps[sub][:, :FGD],
                                        recip[:, sub:sub + 1])
                                    tp2 = bps.tile([128, FGD], F32, tag="sc", bufs=2, name="tp2")
                                    for fp in range(2):
                                        nc.tensor.transpose(
                                            tp2[:, fp * 128:(fp + 1) * 128],
                                            o_sb[:, fp * 128:(fp + 1) * 128],
                                            ident[:])
                                    col = qc * 512 + sub * 128
                                    eng = nc.scalar if sub % 2 else nc.vector
                                    for fp in range(2):
                                        _copy(eng,
                                              ot[(b, hl, fp)][:, col:col + 128],
                                              tp2[:, fp * 128:(fp + 1) * 128])

                # ================= Phase C: output projection =================
                with tc.tile_pool(name="wob", bufs=1) as pwo, \
                     tc.tile_pool(name="cev", bufs=4) as pcev, \
                     tc.tile_pool(name="cpsum", bufs=1, space="PSUM") as cps:
                    for half in range(2):
                        wo_t = pwo.tile([128, 8 * 2048], F32R,
                                         name=f"wo{half}", tag="wo")
                        for j in range(4):
                            for fp in range(2):
                                nc.sync.dma_start(
                                    wo_t[:, (j * 2 + fp) * 2048:
                                         (j * 2 + fp + 1) * 2048],
                                    wo_d[j, fp, :, half * 2048:(half + 1) * 2048])
                        for b in range(B):
                            for tt in range(S // 128):
                                hl = tt // 4
                                for mci in range(4):
                                    mc = half * 4 + mci
                                    ops = cps.tile([128, 512], F32, tag="oc", bufs=4)
                                    for j in range(4):
                                        for fp in range(2):
                                            otr = ot[(b, hl, fp)][:].rearrange(
                                                "p (x s) -> p x s", s=4)
                                            lhsT = otr[:, (tt % 4) * 128:
                                                       (tt % 4 + 1) * 128, j]
                                            rhs = wo_t[:, (j * 2 + fp) * 2048
                                                       + mci * 512:
                                                       (j * 2 + fp) * 2048
                                                       + (mci + 1) * 512]
                                            nc.tensor.matmul(
                                                ops[:], lhsT, rhs,
                                                start=(j == 0 and fp == 0),
                                                stop=(j == 3 and fp == 1))
                                    ev = pcev.tile([128, 512], F32, tag="cev")
                                    eng = nc.scalar if (tt + mci) % 2 else nc.vector
                                    _copy(eng, ev[:], ops[:])
                                    nc.sync.dma_start(
                                        out_d[b, tt * 128:(tt + 1) * 128,
                                              mc * 512:(mc + 1) * 512],
                                        ev[:])

    nc.compile()
    return nc


def _round_f32r(x):
    """Round fp32 array to the fp32r operand format (11 explicit mantissa
    bits, round-to-nearest) so DMAs need no on-the-fly cast."""
    xi = np.ascontiguousarray(x, np.float32).view(np.int32)
    shift = 23 - 11
    bias = 1 << (shift - 1)
    xi = ((xi.astype(np.int64) + bias) >> shift << shift)
    return np.clip(xi, -2**31, 2**31 - 1).astype(np.int32).view(np.float32)


def _host_prep(inputs):
    """Slice/transposes per core; returns (in_maps, fallback_needed)."""
    hs = np.ascontiguousarray(inputs["hidden_states"], dtype=np.float32)
    mask = np.ascontiguousarray(inputs["attention_mask"], dtype=np.float32)
    pos = np.asarray(inputs["position_ids"])
    Wq = np.asarray(inputs["Wq"], dtype=np.float32)
    WVT = np.asarray(inputs["WVT"], dtype=np.float32)
    U = np.asarray(inputs["U"], dtype=np.float32)
    Wv = np.asarray(inputs["Wv"], dtype=np.float32)
    Wo = np.asarray(inputs["Wo"], dtype=np.float32)

    # Verify causal-family mask: strictly-lower 128-blocks all zero,
    # strictly-upper all <= -1e8 (else fall back to numpy reference).
    nt = S // 128
    mb = mask.reshape(B, nt, 128, nt, 128).transpose(0, 1, 3, 2, 4)
    lower_ok = True
    for b in range(B):
        for i in range(nt):
            for k in range(nt):
                blk = mb[b, i, k]
                if k < i and not np.all(blk == 0.0):
                    lower_ok = False
                if k > i and not np.all(blk <= -1e8):
                    lower_ok = False
    if not lower_ok:
        return None, True

    hidT = _round_f32r(hs.transpose(0, 2, 1))                   # (B, HID, S)

    # RoPE tables, 4x head-replicated; sin sign-folded for the rotate-half
    inv = 1.0 / (THETA ** (np.arange(0, D, 2, dtype=np.float32) / D))
    fr = pos.astype(np.float32)[..., None] * inv                # (B, S, 64)
    emb = np.concatenate([fr, fr], axis=-1)                     # (B, S, 128)
    cos = np.cos(emb).astype(np.float32)
    sin = np.sin(emb).astype(np.float32)
    sgn = np.concatenate([-np.ones(64, np.float32), np.ones(64, np.float32)])
    sin_s = sin * sgn
    cos4 = np.ascontiguousarray(np.tile(cos, (1, 1, GS)))       # (B, S, 512)
    sin4 = np.ascontiguousarray(np.tile(sin_s, (1, 1, GS)))

    # Transposed diagonal mask tiles (k, q)
    maskT = np.ascontiguousarray(
        np.stack([np.stack([mask[b, 0, t * 128:(t + 1) * 128,
                                 t * 128:(t + 1) * 128].T
                            for t in range(nt)]) for b in range(B)]))

    scale = np.float32(1.0 / np.sqrt(D))
    in_maps = []
    for g in range(NCORES):
        wqT = _round_f32r(Wq[g * GD:(g + 1) * GD, :].T * scale)
        wkT = _round_f32r(WVT[g * RK:(g + 1) * RK, :].T)
        wvT = _round_f32r(Wv[g * RK:(g + 1) * RK, :].T)
        ugT = _round_f32r(U[:, g * RK:(g + 1) * RK].T)
        woT = np.empty((4, 2, 128, HID), np.float32)
        for j in range(4):
            base = j * 2048 + g * FGD
            blk = Wo[:, base:base + FGD].T                       # (256, 4096)
            woT[j, 0] = blk[:128]
            woT[j, 1] = blk[128:]
        in_maps.append(dict(hidT=hidT, wqT=wqT, wkT=wkT, wvT=wvT, ugT=ugT,
                            cos4=cos4, sin4=sin4, maskT=maskT,
                            woT=_round_f32r(woT)))
    return in_maps, False


def _numpy_fallback(inputs):
    hs = np.asarray(inputs["hidden_states"], np.float32)
    mask = np.asarray(inputs["attention_mask"], np.float32)
    pos = np.asarray(inputs["position_ids"])
    Wq, WVT, U, Wv, Wo = (np.asarray(inputs[k], np.float32)
                          for k in ["Wq", "WVT", "U", "Wv", "Wo"])
    b, q = hs.shape[:2]
    qs = (hs @ Wq.T).reshape(b, q, NH, D).transpose(0, 2, 1, 3)
    klat = (hs @ WVT.T).reshape(b, q, G, RK).transpose(0, 2, 1, 3)
    vlat = (hs @ Wv.T).reshape(b, q, G, FGD).transpose(0, 2, 1, 3)
    Ugr = U.reshape(GD, G, RK)
    keys = np.einsum("bgsr,dgr->bgsd", klat, Ugr)
    keys = keys.transpose(0, 2, 1, 3).reshape(b, q, NH, D).transpose(0, 2, 1, 3)
    inv = 1.0 / (THETA ** (np.arange(0, D, 2, dtype=np.float32) / D))
    fr = pos.astype(np.float32)[..., None] * inv
    emb = np.concatenate([fr, fr], -1)
    cos, sin = np.cos(emb)[:, None], np.sin(emb)[:, None]

    def rot(x):
        return np.concatenate([-x[..., D // 2:], x[..., :D // 2]], -1)
    qs = qs * cos + rot(qs) * sin
    keys = keys * cos + rot(keys) * sin
    att = np.einsum("bhqd,bhkd->bhqk", qs, keys) / np.sqrt(D).astype(np.float32)
    att = att + mask
    att = att - att.max(-1, keepdims=True)
    att = np.exp(att)
    att = att / att.sum(-1, keepdims=True)
    aw = att.reshape(b, G, q * GS, q)
    o = np.einsum("bgik,bgkf->bgif", aw.astype(np.float32),
                  vlat.astype(np.float32))
    o = o.transpose(0, 2, 1, 3).reshape(b, q, 8192)
    return (o @ Wo.T).astype(np.float32)


def _make_timing_fn(nc):
    """Build the sharded jit callable for this Bass module.

    Mirrors bass2jax.run_bass_via_pjrt's multi-core path; returns
    (fn, in_names, out_avals, sharding)."""
    import jax
    from jax.sharding import Mesh, NamedSharding, PartitionSpec
    from jax.experimental.shard_map import shard_map
    from concourse import bass2jax, mybir as _mb

    bass2jax.install_neuronx_cc_hook()

    part_name = (nc.partition_id_tensor.name
                 if nc.partition_id_tensor is not None else None)
    in_names, out_names, out_avals = [], [], []
    for alloc in nc.m.functions[0].allocations:
        if not isinstance(alloc, _mb.MemoryLocationSet):
            continue
        name = alloc.memorylocations[0].name
        if alloc.kind == "ExternalInput":
            if name != part_name:
                in_names.append(name)
        elif alloc.kind == "ExternalOutput":
            out_names.append(name)
            out_avals.append(jax.core.ShapedArray(
                tuple(alloc.tensor_shape), _mb.dt.np(alloc.dtype)))
    n_params = len(in_names)
    all_names = in_names + out_names
    if part_name is not None:
        all_names = all_names + [part_name]

    def _body(*args):
        operands = list(args)
        if part_name is not None:
            operands.append(bass2jax.partition_id_tensor())
        outs = bass2jax._bass_exec_p.bind(
            *operands,
            out_avals=tuple(out_avals),
            in_names=tuple(all_names),
            out_names=tuple(out_names),
            lowering_input_output_aliases=(),
            sim_require_finite=True,
            sim_require_nnan=True,
            nc=nc,
        )
        return tuple(outs)

    devices = jax.devices()[:NCORES]
    mesh = Mesh(np.asarray(devices), ("core",))
    spec = PartitionSpec("core")
    n_outs = len(out_names)
    fn = jax.jit(
        shard_map(_body, mesh=mesh, in_specs=(spec,) * (n_params + n_outs),
                  out_specs=(spec,) * n_outs, check_rep=False),
        keep_unused=True,
    )
    return fn, in_names, out_names, out_avals, NamedSharding(mesh, spec)


def _run_spmd(nc, in_maps, time_iters=0):
    """Execute the SPMD kernel on the first NCORES neuron devices via PJRT.

    Keeps inputs device-resident so repeated executions can be wall-clocked.
    Returns (results_per_core, exec_ns_best or None)."""
    import time as _time

    import jax

    if "timing_fn" not in _NC_CACHE:
        _NC_CACHE["timing_fn"] = _make_timing_fn(nc)
    fn, in_names, out_names, out_avals, sharding = _NC_CACHE["timing_fn"]
    dev_in = [
        jax.device_put(
            np.concatenate([np.asarray(m[name]) for m in in_maps], axis=0),
            sharding)
        for name in in_names
    ]
    dev_zero = [
        jax.device_put(
            np.zeros((NCORES * a.shape[0], *a.shape[1:]), a.dtype), sharding)
        for a in out_avals
    ]
    out = jax.block_until_ready(fn(*dev_in, *dev_zero))

    exec_ns = None
    if time_iters > 0:
        times = []
        for _ in range(time_iters):
            t0 = _time.perf_counter()
            r = jax.block_until_ready(fn(*dev_in, *dev_zero))
            times.append(_time.perf_counter() - t0)
        del r
        exec_ns = int(min(times) * 1e9)
        _NC_CACHE["bench_times"] = times

    results = []
    for c in range(NCORES):
        results.append({
            name: np.asarray(out[i]).reshape(NCORES, *out_avals[i].shape)[c]
            for i, name in enumerate(out_names)
        })
    return results, exec_ns


def kernel(**inputs):
    import os

    in_maps, fallback = _host_prep(inputs)
    if fallback:
        return _numpy_fallback(inputs)

    _install_loud_cc_hook()
    if "nc" not in _NC_CACHE:
        _NC_CACHE["nc"] = _build_nc()
    nc = _NC_CACHE["nc"]

    iters = int(os.environ.get("TRN_KERNEL_TIME_ITERS", "0"))
    results, exec_ns = _run_spmd(nc, in_maps, time_iters=iters)
    _NC_CACHE["last_exec_ns"] = exec_ns

    acc = np.zeros((B, S, HID), np.float64)
    for r in results:
        acc += r["out"].astype(np.float64)
    return acc.astype(np.float32)



# revision 25
# speedup vs baseline: 4.2500x; 4.2500x over previous
"""PALU low-rank Llama attention on 8 Trainium2 NeuronCores.

Tensor-parallel over the 8 PALU groups (1 group = 4 heads per core), with each
core computing a partial contribution to the full output (its group's slice of
the fused Wo contraction); the host sums the 8 partials.

Per-core pipeline (one Bass/Tile kernel, SPMD over 8 cores with per-core
weight shards as inputs):
  A) projections, software-pipelined over 512-token chunks with a 4+4 PSUM
     bank split: pass1 accumulates K-latent (transposed) and V (natural) in
     banks 0-3 while pass2 accumulates Q directly in transposed (d, s) layout
     in banks 4-7.  K is reconstructed transposed from the latent via U.
     RoPE is applied in the transposed layout using a signed half-rotation
     permutation matmul.  V is transposed to natural (s, f') layout with
     cheap bf16 TensorE transposes (one PSUM accumulation group per bank —
     a hardware requirement).  Q.T/K.T spill to DRAM scratch; V stays
     resident in SBUF.
  B) attention as one software-pipelined stream over every (head, q-chunk,
     kt-pair) unit: scores computed transposed S.T = K.T' Q (k on
     partitions) at exact causal widths, exp on ScalarE straight out of
     PSUM (the diagonal mask is applied post-exp as a multiply by
     host-precomputed exp(mask)), then O = P.T' V with an extra all-ones V
     column producing the softmax denominators for free.  PV of unit n
     trails scores+exp of unit n+2 on TensorE so exp latency never stalls
     the PE.  O is normalized with per-partition reciprocal scalars and
     moved into O.T buffers via DMA XBAR transposes (no TensorE, no
     ScalarE — exp keeps the Activation queue).
  C) fused output projection with the exact torch reshape/transpose
     semantics folded into strided access patterns on O.T.

All matmul operands are bf16 (TensorE full rate, half the DMA/SBUF traffic of
fp32); PSUM accumulation is fp32.  Weights/activations are pre-laid-out on the
host so every bulk load is a single contiguous 2D DMA, spread across the
DMA-capable engine queues (SP / Activation / GpSimd).  Measured end-to-end
error vs the fp32 reference is ~1e-3 relative to the output absmax.
"""

import numpy as np
import ml_dtypes

import concourse.bass as bass
import concourse.tile as tile
from concourse import bacc, mybir
from concourse.masks import make_identity
from concourse.bass_utils import run_bass_kernel_spmd

F32 = mybir.dt.float32
BF16 = mybir.dt.bfloat16
NPBF = ml_dtypes.bfloat16
EXP = mybir.ActivationFunctionType.Exp

B, S, HID = 2, 2048, 4096
NH, D = 32, 128
G, GS = 8, 4
RK, RV, FGD, GD = 256, 2048, 256, 512
NCORES = 8
CT = HID // 128        # 32 contraction tiles over hidden dim
NQC = S // 512         # 4 q-chunks of 512
THETA = 10000.0

_NC_CACHE = {}


def _install_loud_cc_hook():
    """Surface exceptions thrown inside the neuronx_cc compile hook (the C++
    callback boundary otherwise swallows them into an opaque INTERNAL error)."""
    if _NC_CACHE.get("loud_hook"):
        return
    import traceback
    from concourse import bass2jax
    orig = bass2jax.neuronx_cc_hook

    def loud_hook(*a, **kw):
        try:
            return orig(*a, **kw)
        except BaseException:
            traceback.print_exc()
            raise

    bass2jax.neuronx_cc_hook = loud_hook
    _NC_CACHE["loud_hook"] = True


def _copy(eng_nc, dst, src):
    """Engine-agnostic copy: ScalarE uses activation-Copy, others tensor_copy."""
    if hasattr(eng_nc, "tensor_copy"):
        eng_nc.tensor_copy(dst, src)
    else:
        eng_nc.copy(dst, src)


def _build_nc():
    """Build + compile the per-core Bass kernel (same NEFF for all cores)."""
    nc = bacc.Bacc(trn_type="TRN2", target_bir_lowering=False, debug=False)

    # host-prearranged inputs (see _host_prep): every bulk load is one 2D DMA
    hid_d = nc.dram_tensor("hid2", [B, NQC, 128, CT * 512], BF16,
                           kind="ExternalInput").ap()
    wq_d = nc.dram_tensor("wq2", [128, CT * GD], BF16, kind="ExternalInput").ap()
    wk_d = nc.dram_tensor("wk2", [128, CT * RK], BF16, kind="ExternalInput").ap()
    wv_d = nc.dram_tensor("wv2", [128, CT * FGD], BF16, kind="ExternalInput").ap()
    ug_d = nc.dram_tensor("ug2", [128, 2 * GD], BF16, kind="ExternalInput").ap()
    cosT_d = nc.dram_tensor("cosT", [B, 128, S], BF16, kind="ExternalInput").ap()
    sinT_d = nc.dram_tensor("sinT", [B, 128, S], BF16, kind="ExternalInput").ap()
    perm_d = nc.dram_tensor("perm", [128, 128], BF16, kind="ExternalInput").ap()
    msk_d = nc.dram_tensor("msk2", [B, 128, (S // 128) * 128], BF16,
                           kind="ExternalInput").ap()
    wo_d = nc.dram_tensor("wo2", [2, 128, 8 * 2048], BF16,
                          kind="ExternalInput").ap()
    out_d = nc.dram_tensor("out", [B, S, HID], F32, kind="ExternalOutput").ap()

    # DRAM scratch (per-core internal)
    qT_s = nc.dram_tensor("qT_s", [B, GS, 128, S], BF16).ap()
    kT_s = nc.dram_tensor("kT_s", [B, GS, 128, S], BF16).ap()

    with tile.TileContext(nc) as tc:
        # ---------- constants + tensors persisting across phases ----------
        with tc.tile_pool(name="const", bufs=1) as pc:
            perm_id = pc.tile([128, 128], BF16, tag="ident")
            make_identity(nc, perm_id)
            perm_sb = pc.tile([128, 128], BF16, tag="perm")
            m_all = [pc.tile([128, (S // 128) * 128], BF16, name=f"mall{b}",
                             tag=f"mall{b}") for b in range(B)]
            v_hold = {}
            for b in range(B):
                for kt in range(S // 128):
                    v_hold[(b, kt)] = pc.tile(
                        [128, FGD + 2], BF16, name=f"vh{b}_{kt}",
                        tag=f"vh{b}_{kt}")

            # kt/qt reload pools opened around phase A so loads can be
            # emitted (and run) as soon as each batch's spills complete
            pkt = tc.alloc_tile_pool(name="ktb", bufs=4)
            pqt = tc.alloc_tile_pool(name="qtb", bufs=3)
            kt_tiles, qt_tiles = {}, {}
            qt_seq = [(i, qc) for i in range(B * GS) for qc in range(NQC)]

            def load_kt(i):
                bb, hl = divmod(i, GS)
                tkt = pkt.tile([128, S], BF16, tag="kt", name=f"kt{i}")
                nc.gpsimd.dma_start(tkt[:], kT_s[bb, hl])
                kt_tiles[i] = tkt

            def load_qt(pos):
                i, qc = qt_seq[pos]
                bb, hl = divmod(i, GS)
                tqt = pqt.tile([128, 512], BF16, tag="qt", name=f"qt{i}_{qc}")
                nc.sync.dma_start(tqt[:],
                                  qT_s[bb, hl, :, qc * 512:(qc + 1) * 512])
                qt_tiles[(i, qc)] = tqt

            # ================= Phase A: projections =================
            with tc.tile_pool(name="wts", bufs=1) as pw, \
                 tc.tile_pool(name="ht", bufs=2) as pht, \
                 tc.tile_pool(name="tab", bufs=1) as ptab, \
                 tc.tile_pool(name="stg", bufs=3) as pstg, \
                 tc.tile_pool(name="xnb", bufs=4) as pxn, \
                 tc.tile_pool(name="apsum", bufs=1, space="PSUM") as pps:

                # weight loads: 4 block-DMAs each, spread across engine queues
                wq_sb = pw.tile([128, CT * GD], BF16, tag="wq")
                wk_sb = pw.tile([128, CT * RK], BF16, tag="wk")
                wv_sb = pw.tile([128, CT * FGD], BF16, tag="wv")
                ug_sb = pw.tile([128, 2 * GD], BF16, tag="ug")
                wblks = [(0, 2), (2, 8), (8, 16), (16, 24), (24, 32)]
                for c0, c1 in wblks:
                    nc.scalar.dma_start(wq_sb[:, c0 * GD:c1 * GD],
                                        wq_d[:, c0 * GD:c1 * GD])
                    nc.gpsimd.dma_start(wk_sb[:, c0 * RK:c1 * RK],
                                        wk_d[:, c0 * RK:c1 * RK])
                    nc.gpsimd.dma_start(wv_sb[:, c0 * FGD:c1 * FGD],
                                        wv_d[:, c0 * FGD:c1 * FGD])
                nc.scalar.dma_start(ug_sb[:], ug_d[:])
                # const loads/memsets after the weight DMAs so they don't
                # delay the first pass1 matmuls on the gpsimd queue
                nc.gpsimd.dma_start(perm_sb[:], perm_d[:])
                for b in range(B):
                    nc.gpsimd.dma_start(m_all[b][:], msk_d[b])
                for key in v_hold:
                    nc.gpsimd.memset(v_hold[key][:, FGD:FGD + 2], 1.0)

                def rope_T(sb_h, rot_ps, cs, sn, dst_dram, eng):
                    """RoPE in transposed (d, s) layout.

                    sb_h: SBUF (128, 512) bf16 pre-rotation head tile.
                    rot_ps: PSUM (128, 512) f32 = perm @ sb_h (signed half
                    rotation).  out = sb_h * cs + rot_ps * sn -> bf16, spilled
                    to dst_dram."""
                    t1 = pstg.tile([128, 512], BF16, tag="t1")
                    t2 = pstg.tile([128, 512], BF16, tag="t2")
                    eng.tensor_mul(t1[:], sb_h[:], cs)
                    # GPSIMD cannot touch PSUM on hw: rot_ps read stays on DVE
                    nc.vector.tensor_mul(t2[:], rot_ps[:], sn)
                    xn = pxn.tile([128, 512], BF16, tag="xn")
                    eng.tensor_add(xn[:], t1[:], t2[:])
                    nc.sync.dma_start(dst_dram, xn[:])

                for b in range(B):
                    cosT_t = ptab.tile([128, S], BF16, tag="cos")
                    sinT_t = ptab.tile([128, S], BF16, tag="sin")
                    nc.scalar.dma_start(cosT_t[:], cosT_d[b])
                    nc.scalar.dma_start(sinT_t[:], sinT_d[b])
                    for qc in range(NQC):
                        # hidden chunk: 4 block DMAs into one big tile
                        t = pht.tile([128, CT * 512], BF16, tag="ht")
                        hblks = ([(0, 2), (2, 8)] if (b, qc) == (0, 0)
                                 else [(0, 8)]) + [(8, 16), (16, 24), (24, 32)]
                        for c0, c1 in hblks:
                            nc.sync.dma_start(
                                t[:, c0 * 512:c1 * 512],
                                hid_d[b, qc, :, c0 * 512:c1 * 512])
                        cs = cosT_t[:, qc * 512:(qc + 1) * 512]
                        sn = sinT_t[:, qc * 512:(qc + 1) * 512]

                        # ---- pass1: K-latent.T + V natural (PSUM banks 0-3)
                        klp = [pps.tile([128, 512], F32, tag="klp", bufs=2,
                                        name=f"klp{rt}") for rt in range(2)]
                        vp = [pps.tile([128, 512], F32, tag="vnp", bufs=2,
                                       name=f"vp{i}") for i in range(2)]
                        for ct in range(CT):
                            tt = t[:, ct * 512:(ct + 1) * 512]
                            st, sp = (ct == 0), (ct == CT - 1)
                            for rt in range(2):
                                nc.tensor.matmul(
                                    klp[rt][:],
                                    wk_sb[:, ct * RK + rt * 128:
                                          ct * RK + (rt + 1) * 128],
                                    tt, start=st, stop=sp)
                            # V transposed (f'-tile, 512 q): one PSUM
                            # accumulation group per bank (hw requirement)
                            for fp in range(2):
                                nc.tensor.matmul(
                                    vp[fp][:],
                                    wv_sb[:, ct * FGD + fp * 128:
                                          ct * FGD + (fp + 1) * 128],
                                    tt, start=st, stop=sp)

                        # ---- pass2: Q.T per head (PSUM banks 4-7)
                        qtp = [pps.tile([128, 512], F32, tag="qtp", bufs=4,
                                        name=f"qtp{h}") for h in range(GS)]
                        for ct in range(CT):
                            tt = t[:, ct * 512:(ct + 1) * 512]
                            st, sp = (ct == 0), (ct == CT - 1)
                            for h in range(GS):
                                nc.tensor.matmul(
                                    qtp[h][:],
                                    wq_sb[:, ct * GD + h * 128:
                                          ct * GD + (h + 1) * 128],
                                    tt, start=st, stop=sp)

                        # ---- evacuations (overlap pass2 on other engines)
                        klat_sb = []
                        for rt in range(2):
                            ksb = pstg.tile([128, 512], BF16, tag="klsb",
                                            name=f"ksb{rt}")
                            nc.scalar.copy(ksb[:], klp[rt][:])
                            klat_sb.append(ksb)
                        vt_sb = []
                        for fp in range(2):
                            vtsb = pstg.tile([128, 512], BF16, tag="vtsb",
                                             name=f"vtsb{fp}")
                            eng = nc.scalar if fp else nc.vector
                            _copy(eng, vtsb[:], vp[fp][:])
                            vt_sb.append(vtsb)
                        # transpose V.T -> natural (q, f) and park in v_hold
                        for i in range(2):
                            vnT = pps.tile([128, 512], BF16, tag="vnp",
                                           bufs=2, name=f"vnT{i}")
                            for half in range(2):
                                sub = 2 * i + half
                                for fp in range(2):
                                    nc.tensor.matmul(
                                        vnT[:, half * FGD + fp * 128:
                                            half * FGD + (fp + 1) * 128],
                                        vt_sb[fp][:, sub * 128:
                                                   (sub + 1) * 128],
                                        perm_id[:], is_transpose=True,
                                        start=True, stop=True)
                            for half in range(2):
                                sub = 2 * i + half
                                eng = nc.scalar if half else nc.vector
                                _copy(eng,
                                      v_hold[(b, qc * 4 + sub)][:, :FGD],
                                      vnT[:, half * FGD:(half + 1) * FGD])

                        # ---- K.T reconstruction + rope (banks from klp tag)
                        k_sb = []
                        for h in range(GS):
                            kp = pps.tile([128, 512], F32, tag="klp", bufs=2,
                                          name=f"kTp{h}")
                            for rt in range(2):
                                nc.tensor.matmul(
                                    kp[:],
                                    ug_sb[:, rt * GD + h * 128:
                                          rt * GD + (h + 1) * 128],
                                    klat_sb[rt][:],
                                    start=(rt == 0), stop=(rt == 1))
                            ksb_h = pstg.tile([128, 512], BF16, tag="khsb")
                            nc.scalar.copy(ksb_h[:], kp[:])
                            k_sb.append(ksb_h)
                        # Q evacuations on DVE (parallel with scalar K evacs)
                        q_sb = []
                        for h in range(GS):
                            qsb_h = pstg.tile([128, 512], BF16, tag="qhsb")
                            if h % 2:
                                nc.scalar.copy(qsb_h[:], qtp[h][:])
                            else:
                                nc.vector.tensor_copy(qsb_h[:], qtp[h][:])
                            q_sb.append(qsb_h)
                        # signed half-rotation perm matmuls + rope + spill
                        for h in range(GS):
                            rot = pps.tile([128, 512], F32, tag="vnp", bufs=2,
                                           name=f"rotk{h}")
                            nc.tensor.matmul(rot[:], perm_sb[:], k_sb[h][:],
                                             start=True, stop=True)
                            rope_T(k_sb[h], rot, cs, sn,
                                   kT_s[b, h, :, qc * 512:(qc + 1) * 512],
                                   nc.gpsimd if h % 2 else nc.vector)
                        for h in range(GS):
                            rot = pps.tile([128, 512], F32, tag="vnp", bufs=2,
                                           name=f"rotq{h}")
                            nc.tensor.matmul(rot[:], perm_sb[:], q_sb[h][:],
                                             start=True, stop=True)
                            rope_T(q_sb[h], rot, cs, sn,
                                   qT_s[b, h, :, qc * 512:(qc + 1) * 512],
                                   nc.vector if h % 2 else nc.gpsimd)
                    if b == 0:
                        for hl in range(GS):
                            load_kt(hl)
                        load_qt(0)
                        load_qt(1)

            # ================= Phase B: attention =================
            with tc.tile_pool(name="otb", bufs=1) as potb, \
                 tc.tile_pool(name="wob", bufs=1) as pwo:
                # prefetch full Wo (bf16) while attention runs
                wo_t = []
                for half in range(2):
                    w = pwo.tile([128, 8 * 2048], BF16, name=f"wo{half}",
                                 tag=f"wo{half}")
                    for piece in range(2):
                        nc.gpsimd.dma_start(
                            w[:, piece * 8192:(piece + 1) * 8192],
                            wo_d[half][:, piece * 8192:(piece + 1) * 8192])
                    wo_t.append(w)

                ot = {}
                for b in range(B):
                    for hl in range(GS):
                        for fp in range(2):
                            ot[(b, hl, fp)] = potb.tile(
                                [128, S], BF16, name=f"ot{b}{hl}{fp}",
                                tag=f"ot{b}{hl}{fp}")

                with tc.tile_pool(name="ptb", bufs=3) as ppt, \
                     tc.tile_pool(name="bsm", bufs=4) as pbs, \
                     tc.tile_pool(name="bpsum", bufs=1, space="PSUM") as bps:

                    # one software-pipelined stream over every
                    # (head, q-chunk, kt-pair) unit: PV of unit n trails the
                    # scores+exp of unit n+1 on TensorE, across qc and head
                    # boundaries alike, so exp latency and o_ps bank handover
                    # never stall the PE.
                    units = []
                    for i in range(B * GS):
                        for qc in range(NQC):
                            for g2 in range(0, 4 * qc + 4, 2):
                                units.append((i, qc, g2))

                    ops_map = {}

                    def emit_scores(i, qc, g2):
                        b, hl = divmod(i, GS)
                        kt_sb = kt_tiles[i]
                        qt_sb = qt_tiles[(i, qc)]
                        kts = [g2, g2 + 1]
                        sc = bps.tile([128, 1024], F32, tag="sc", bufs=2)
                        offs = []
                        for k, kt in enumerate(kts):
                            off = max(0, (kt - 4 * qc) * 128)
                            offs.append(off)
                            nc.tensor.matmul(
                                sc[:, k * 512 + off:(k + 1) * 512],
                                kt_sb[:, kt * 128:(kt + 1) * 128],
                                qt_sb[:, off:512],
                                start=True, stop=True)
                        pt = ppt.tile([128, 1024], BF16, tag="pt")
                        if offs == [0, 0]:
                            nc.scalar.activation(pt[:], sc[:], EXP)
                        else:
                            for k in range(2):
                                nc.scalar.activation(
                                    pt[:, k * 512 + offs[k]:(k + 1) * 512],
                                    sc[:, k * 512 + offs[k]:(k + 1) * 512],
                                    EXP)
                        ptd = {}
                        for k, kt in enumerate(kts):
                            if kt >= 4 * qc:
                                sub = kt - 4 * qc
                                pd = pbs.tile([128, 128], BF16, tag="ptd")
                                nc.vector.tensor_mul(
                                    pd[:],
                                    pt[:, k * 512 + sub * 128:
                                       k * 512 + (sub + 1) * 128],
                                    m_all[b][:, kt * 128:(kt + 1) * 128])
                                ptd[kt] = pd
                        return pt, ptd

                    def emit_pv(i, qc, g2, pt, ptd):
                        b, hl = divmod(i, GS)
                        o_ps = ops_map[(i, qc)]
                        for k, kt in enumerate([g2, g2 + 1]):
                            for sub in range(4):
                                if kt <= 4 * qc + sub:
                                    if kt == 4 * qc + sub:
                                        lhs = ptd[kt][:]
                                    else:
                                        lhs = pt[:, k * 512 + sub * 128:
                                                 k * 512 + (sub + 1) * 128]
                                    nc.tensor.matmul(
                                        o_ps[sub][:],
                                        lhs,
                                        v_hold[(b, kt)][:],
                                        start=(kt == 0),
                                        stop=(kt == 4 * qc + sub))

                    def emit_evac(i, qc, subs):
                        b, hl = divmod(i, GS)
                        o_ps = ops_map[(i, qc)]
                        recip = pbs.tile([128, 2], F32, tag="rc")
                        for k, sub in enumerate(subs):
                            nc.vector.reciprocal(
                                recip[:, k:k + 1],
                                o_ps[sub][:, FGD:FGD + 1])
                        for k, sub in enumerate(subs):
                            o_sb = pbs.tile([128, FGD], BF16, tag="osb")
                            nc.vector.tensor_scalar_mul(
                                o_sb[:], o_ps[sub][:, :FGD],
                                recip[:, k:k + 1])
                            col = qc * 512 + sub * 128
                            for fp in range(2):
                                nc.sync.dma_start_transpose(
                                    ot[(b, hl, fp)][:, col:col + 128],
                                    o_sb[:, fp * 128:(fp + 1) * 128])

                    from collections import deque
                    pending = deque()

                    def retire(unit):
                        emit_pv(*unit)
                        last = 4 * unit[1] + 4
                        if unit[2] + 2 == last - 2:
                            emit_evac(unit[0], unit[1], (0, 1))
                        elif unit[2] + 2 >= last:
                            emit_evac(unit[0], unit[1], (2, 3))
                            ops_map.pop((unit[0], unit[1]))

                    for u, (i, qc, g2) in enumerate(units):
                        if g2 == 0:
                            if qc == 0 and i < 4:
                                load_kt(i + 4)
                            pos = i * NQC + qc
                            if pos + 2 < len(qt_seq):
                                load_qt(pos + 2)
                            ops_map[(i, qc)] = [
                                bps.tile([128, FGD + 2], F32, tag="ob",
                                         bufs=4, name=f"ob{k}")
                                for k in range(4)]
                        pending.append((i, qc, g2, *emit_scores(i, qc, g2)))
                        if len(pending) > 2:
                            retire(pending.popleft())
                    while pending:
                        retire(pending.popleft())

                # ================= Phase C: output projection =================
                with tc.tile_pool(name="cev", bufs=2) as pcev, \
                     tc.tile_pool(name="cpsum", bufs=1, space="PSUM") as cps:
                    for half in range(2):
                        for b in range(B):
                            for tt in range(S // 128):
                                hl = tt // 4
                                ev = pcev.tile([128, 2048], F32, tag="cev")
                                ops4 = [cps.tile([128, 512], F32, tag="oc",
                                                 bufs=8, name=f"oc{m}")
                                        for m in range(4)]
                                for j in range(4):
                                    for fp in range(2):
                                        otr = ot[(b, hl, fp)][:].rearrange(
                                            "p (x s) -> p x s", s=4)
                                        lhsT = otr[:, (tt % 4) * 128:
                                                   (tt % 4 + 1) * 128, j]
                                        for mci in range(4):
                                            rhs = wo_t[half][
                                                :, (j * 2 + fp) * 2048
                                                + mci * 512:
                                                (j * 2 + fp) * 2048
                                                + (mci + 1) * 512]
                                            nc.tensor.matmul(
                                                ops4[mci][:], lhsT, rhs,
                                                start=(j == 0 and fp == 0),
                                                stop=(j == 3 and fp == 1))
                                for mci in range(4):
                                    eng = nc.scalar if (tt + mci) % 2 else nc.vector
                                    _copy(eng, ev[:, mci * 512:(mci + 1) * 512],
                                          ops4[mci][:])
                                nc.sync.dma_start(
                                    out_d[b, tt * 128:(tt + 1) * 128,
                                          half * 2048:(half + 1) * 2048],
                                    ev[:])

            pqt.release()
            pkt.release()

    nc.compile()
    return nc


def _host_prep(inputs):
    """Slice/transposes per core; returns (in_maps, fallback_needed)."""
    hs = np.ascontiguousarray(inputs["hidden_states"], dtype=np.float32)
    mask = np.ascontiguousarray(inputs["attention_mask"], dtype=np.float32)
    pos = np.asarray(inputs["position_ids"])
    Wq = np.asarray(inputs["Wq"], dtype=np.float32)
    WVT = np.asarray(inputs["WVT"], dtype=np.float32)
    U = np.asarray(inputs["U"], dtype=np.float32)
    Wv = np.asarray(inputs["Wv"], dtype=np.float32)
    Wo = np.asarray(inputs["Wo"], dtype=np.float32)

    # Verify causal-family mask: strictly-lower 128-blocks all zero,
    # strictly-upper all <= -1e8 (else fall back to numpy reference).
    nt = S // 128
    mb = mask.reshape(B, nt, 128, nt, 128).transpose(0, 1, 3, 2, 4)
    lower_ok = True
    for b in range(B):
        for i in range(nt):
            for k in range(nt):
                blk = mb[b, i, k]
                if k < i and not np.all(blk == 0.0):
                    lower_ok = False
                if k > i and not np.all(blk <= -1e8):
                    lower_ok = False
    if not lower_ok:
        return None, True

    # hidden: (B, S, HID) -> (B, NQC, 128, CT*512) with
    # hid2[b, qc, p, ct*512+s] = hs[b, qc*512+s, ct*128+p]
    hid2 = np.ascontiguousarray(
        hs.reshape(B, NQC, 512, CT, 128).transpose(0, 1, 4, 3, 2)
        .reshape(B, NQC, 128, CT * 512)).astype(NPBF)

    # RoPE tables in transposed (d, s) layout; sign of the half-rotation is
    # folded into the permutation matrix.
    inv = 1.0 / (THETA ** (np.arange(0, D, 2, dtype=np.float32) / D))
    fr = pos.astype(np.float32)[..., None] * inv                # (B, S, 64)
    emb = np.concatenate([fr, fr], axis=-1)                     # (B, S, 128)
    cosT = np.ascontiguousarray(
        np.cos(emb).transpose(0, 2, 1)).astype(NPBF)            # (B, 128, S)
    sinT = np.ascontiguousarray(
        np.sin(emb).transpose(0, 2, 1)).astype(NPBF)
    # perm[k, m]: rot(x).T[m] = sum_k perm[k, m] * x.T[k]
    #   m <  64: rot[m] = -x[m+64]  -> perm[m+64, m] = -1
    #   m >= 64: rot[m] = +x[m-64]  -> perm[m-64, m] = +1
    perm = np.zeros((128, 128), np.float32)
    for m in range(64):
        perm[m + 64, m] = -1.0
        perm[m, m + 64] = 1.0
    perm = perm.astype(NPBF)

    # exp of transposed diagonal mask tiles (k, q), packed (B, 128, nt*128):
    # msk2[b, p, t*128+c] = exp(mask[b, 0, t*128+c, t*128+p]); applied as a
    # post-exp multiply on the diagonal probability subtiles.
    msk2 = np.empty((B, 128, nt * 128), np.float32)
    for b in range(B):
        for t in range(nt):
            msk2[b, :, t * 128:(t + 1) * 128] = np.exp(
                mask[b, 0, t * 128:(t + 1) * 128, t * 128:(t + 1) * 128].T)
    msk2 = np.ascontiguousarray(msk2).astype(NPBF)

    scale = np.float32(1.0 / np.sqrt(D))
    in_maps = []
    for g in range(NCORES):
        # weights laid out so SBUF tile cols match DRAM cols directly:
        # wq2[p, ct*GD+c] = (Wq_g.T * scale)[ct*128+p, c]
        wqT = (Wq[g * GD:(g + 1) * GD, :].T * scale)            # (HID, GD)
        wq2 = np.ascontiguousarray(
            wqT.reshape(CT, 128, GD).transpose(1, 0, 2)
            .reshape(128, CT * GD)).astype(NPBF)
        wkT = WVT[g * RK:(g + 1) * RK, :].T                     # (HID, RK)
        wk2 = np.ascontiguousarray(
            wkT.reshape(CT, 128, RK).transpose(1, 0, 2)
            .reshape(128, CT * RK)).astype(NPBF)
        wvT = Wv[g * RK:(g + 1) * RK, :].T                      # (HID, FGD)
        wv2 = np.ascontiguousarray(
            wvT.reshape(CT, 128, FGD).transpose(1, 0, 2)
            .reshape(128, CT * FGD)).astype(NPBF)
        ugT = U[:, g * RK:(g + 1) * RK].T                       # (RK, GD)
        ug2 = np.ascontiguousarray(
            ugT.reshape(2, 128, GD).transpose(1, 0, 2)
            .reshape(128, 2 * GD)).astype(NPBF)
        # wo2[half, p, (j*2+fp)*2048 + c] = Wo[half*2048+c, j*2048+g*FGD
        #                                       + fp*128 + p]
        wo2 = np.empty((2, 128, 8 * 2048), np.float32)
        for j in range(4):
            base = j * 2048 + g * FGD
            blk = Wo[:, base:base + FGD].T                      # (256, 4096)
            for half in range(2):
                for fp in range(2):
                    wo2[half, :, (j * 2 + fp) * 2048:(j * 2 + fp + 1) * 2048] \
                        = blk[fp * 128:(fp + 1) * 128,
                              half * 2048:(half + 1) * 2048]
        in_maps.append(dict(hid2=hid2, wq2=wq2, wk2=wk2, wv2=wv2, ug2=ug2,
                            cosT=cosT, sinT=sinT, perm=perm, msk2=msk2,
                            wo2=np.ascontiguousarray(wo2).astype(NPBF)))
    return in_maps, False


def _numpy_fallback(inputs):
    hs = np.asarray(inputs["hidden_states"], np.float32)
    mask = np.asarray(inputs["attention_mask"], np.float32)
    pos = np.asarray(inputs["position_ids"])
    Wq, WVT, U, Wv, Wo = (np.asarray(inputs[k], np.float32)
                          for k in ["Wq", "WVT", "U", "Wv", "Wo"])
    b, q = hs.shape[:2]
    qs = (hs @ Wq.T).reshape(b, q, NH, D).transpose(0, 2, 1, 3)
    klat = (hs @ WVT.T).reshape(b, q, G, RK).transpose(0, 2, 1, 3)
    vlat = (hs @ Wv.T).reshape(b, q, G, FGD).transpose(0, 2, 1, 3)
    Ugr = U.reshape(GD, G, RK)
    keys = np.einsum("bgsr,dgr->bgsd", klat, Ugr)
    keys = keys.transpose(0, 2, 1, 3).reshape(b, q, NH, D).transpose(0, 2, 1, 3)
    inv = 1.0 / (THETA ** (np.arange(0, D, 2, dtype=np.float32) / D))
    fr = pos.astype(np.float32)[..., None] * inv
    emb = np.concatenate([fr, fr], -1)
    cos, sin = np.cos(emb)[:, None], np.sin(emb)[:, None]

    def rot(x):
        return np.concatenate([-x[..., D // 2:], x[..., :D // 2]], -1)
    qs = qs * cos + rot(qs) * sin
    keys = keys * cos + rot(keys) * sin
    att = np.einsum("bhqd,bhkd->bhqk", qs, keys) / np.sqrt(D).astype(np.float32)
    att = att + mask
    att = att - att.max(-1, keepdims=True)
    att = np.exp(att)
    att = att / att.sum(-1, keepdims=True)
    aw = att.reshape(b, G, q * GS, q)
    o = np.einsum("bgik,bgkf->bgif", aw.astype(np.float32),
                  vlat.astype(np.float32))
    o = o.transpose(0, 2, 1, 3).reshape(b, q, 8192)
    return (o @ Wo.T).astype(np.float32)


def _make_timing_fn(nc):
    """Build the sharded jit callable for this Bass module.

    Mirrors bass2jax.run_bass_via_pjrt's multi-core path; returns
    (fn, in_names, out_avals, sharding)."""
    import jax
    from jax.sharding import Mesh, NamedSharding, PartitionSpec
    from jax.experimental.shard_map import shard_map
    from concourse import bass2jax, mybir as _mb

    bass2jax.install_neuronx_cc_hook()

    part_name = (nc.partition_id_tensor.name
                 if nc.partition_id_tensor is not None else None)
    in_names, out_names, out_avals = [], [], []
    for alloc in nc.m.functions[0].allocations:
        if not isinstance(alloc, _mb.MemoryLocationSet):
            continue
        name = alloc.memorylocations[0].name
        if alloc.kind == "ExternalInput":
            if name != part_name:
                in_names.append(name)
        elif alloc.kind == "ExternalOutput":
            out_names.append(name)
            out_avals.append(jax.core.ShapedArray(
                tuple(alloc.tensor_shape), _mb.dt.np(alloc.dtype)))
    n_params = len(in_names)
    all_names = in_names + out_names
    if part_name is not None:
        all_names = all_names + [part_name]

    def _body(*args):
        operands = list(args)
        if part_name is not None:
            operands.append(bass2jax.partition_id_tensor())
        outs = bass2jax._bass_exec_p.bind(
            *operands,
            out_avals=tuple(out_avals),
            in_names=tuple(all_names),
            out_names=tuple(out_names),
            lowering_input_output_aliases=(),
            sim_require_finite=True,
            sim_require_nnan=True,
            nc=nc,
        )
        return tuple(outs)

    devices = jax.devices()[:NCORES]
    mesh = Mesh(np.asarray(devices), ("core",))
    spec = PartitionSpec("core")
    n_outs = len(out_names)
    fn = jax.jit(
        shard_map(_body, mesh=mesh, in_specs=(spec,) * (n_params + n_outs),
                  out_specs=(spec,) * n_outs, check_rep=False),
        keep_unused=True,
    )
    return fn, in_names, out_names, out_avals, NamedSharding(mesh, spec)


def _run_spmd(nc, in_maps, time_iters=0):
    """Execute the SPMD kernel on the first NCORES neuron devices via PJRT.

    Keeps inputs device-resident so repeated executions can be wall-clocked.
    Returns (results_per_core, exec_ns_best or None)."""
    import time as _time

    import jax

    if "timing_fn" not in _NC_CACHE:
        _NC_CACHE["timing_fn"] = _make_timing_fn(nc)
    fn, in_names, out_names, out_avals, sharding = _NC_CACHE["timing_fn"]
    dev_in = [
        jax.device_put(
            np.concatenate([np.asarray(m[name]) for m in in_maps], axis=0),
            sharding)
        for name in in_names
    ]
    dev_zero = [
        jax.device_put(
            np.zeros((NCORES * a.shape[0], *a.shape[1:]), a.dtype), sharding)
        for a in out_avals
    ]
    out = jax.block_until_ready(fn(*dev_in, *dev_zero))

    exec_ns = None
    if time_iters > 0:
        times = []
        for _ in range(time_iters):
            t0 = _time.perf_counter()
            r = jax.block_until_ready(fn(*dev_in, *dev_zero))
            times.append(_time.perf_counter() - t0)
        del r
        exec_ns = int(min(times) * 1e9)
        _NC_CACHE["bench_times"] = times

    results = []
    for c in range(NCORES):
        results.append({
            name: np.asarray(out[i]).reshape(NCORES, *out_avals[i].shape)[c]
            for i, name in enumerate(out_names)
        })
    return results, exec_ns


def kernel(**inputs):
    import os

    in_maps, fallback = _host_prep(inputs)
    if fallback:
        return _numpy_fallback(inputs)

    _install_loud_cc_hook()
    if "nc" not in _NC_CACHE:
        _NC_CACHE["nc"] = _build_nc()
    nc = _NC_CACHE["nc"]

    iters = int(os.environ.get("TRN_KERNEL_TIME_ITERS", "0"))
    results, exec_ns = _run_spmd(nc, in_maps, time_iters=iters)
    _NC_CACHE["last_exec_ns"] = exec_ns

    acc = np.zeros((B, S, HID), np.float64)
    for r in results:
        acc += r["out"].astype(np.float64)
    return acc.astype(np.float32)
